# revision 1
# baseline (speedup 1.0000x reference)
"""Trainium2 Bass kernel for nn_DoubleStream_Expert (dense double-stream DiT block).

Sharding (8 cores, no collectives): core c -> batch b = c//4, rank r = c%4.
Each core computes the full K/V projections for its batch (2048 tokens, both
streams), but Q / attention / out-proj / MLP only for its own 512 tokens.
Host slices inputs per core and reassembles the two output streams.

Token chunks are fed in a per-core "slot" order (own chunk, other chunk of my
stream, the two chunks of the other stream) so the SPMD program is identical
across cores; attention is permutation-invariant in keys, and RoPE tables are
permuted on the host to match.

Head dims are padded 96->128 with the rotary halves at rows 0..47 / 64..111,
making rotate_half a uniform +-64 partition move (32-aligned starts, written
via shifted-output ops). Padded weight columns are zero.

Precision: fp32r matmuls (full PE rate at N>=256) for Q/K projections and the
out-projection; bf16 for K/Q storage + rope, probs x V, and the MLP; fp32 for
all softmax statistics, norms and residuals. Softmax needs no running max:
QK-norm bounds |logits| <= max(qk_scale)^2/sqrt(dh).
"""

import numpy as np

import concourse.bass as bass  # noqa: F401
import concourse.mybir as mybir
import concourse.tile as tile
from concourse import bacc
from concourse.bass_utils import run_bass_kernel_spmd
from concourse.masks import make_identity

try:
    import ml_dtypes
    _BF16 = ml_dtypes.bfloat16
except ImportError:  # pragma: no cover
    _BF16 = np.float32

F32 = mybir.dt.float32
F32R = mybir.dt.float32r
BF16 = mybir.dt.bfloat16
AF = mybir.ActivationFunctionType
ALU = mybir.AluOpType

B, T, D, H, DH, MLPD = 2, 1024, 768, 8, 96, 3072
N = 2 * T
NC = 8
CH = 512
KT = D // 128        # 6
MT2 = MLPD // 128    # 24
PH = 128
VW = H * 97          # 776
EPS = 1e-6

_ROWS_LO = np.arange(0, 48)
_ROWS_HI = np.arange(64, 112)

_CACHED = {}


def _bc3(ap2d, nh):
    """[P, C] -> [P, nh, C] stride-0 broadcast over a middle axis."""
    return ap2d.unsqueeze(1).broadcast_to([ap2d.shape[0], nh, ap2d.shape[1]])


def _build():
    if "nc" in _CACHED:
        return _CACHED["nc"]

    nc = bacc.Bacc("TRN2", target_bir_lowering=False, debug=False, num_devices=NC)

    def din(name, shape, dt=F32R):
        return nc.dram_tensor(name, list(shape), dt, kind="ExternalInput").ap()

    x_own = din("x_own", [CH, D], F32)
    x_rest = din("x_rest", [3, CH, D], F32)
    x_own2 = din("x_own2", [CH, D], F32)               # second copy for the residual
    p_my = din("p_my", [1, 1024], BF16)
    mod_w1 = din("mod_w1", [1024, 512], BF16)
    mod_b1 = din("mod_b1", [128, 4], F32)
    mod_w2m = din("mod_w2m", [512, 6 * D], BF16)  # ms_my mh_my ms_ot mh_ot m3s m3h
    mod_b2m = din("mod_b2m", [128, 36], F32)
    mod_w2g = din("mod_w2g", [512, 2 * D], BF16)  # g_my, m3g
    mod_b2g = din("mod_b2g", [1, 2 * D], F32)
    norm1_my = din("norm1_my", [128, KT], F32)
    norm1_ot = din("norm1_ot", [128, KT], F32)
    norm2_my = din("norm2_my", [128, KT], F32)
    wq = din("wq", [D, H * PH])
    bq = din("bq", [128, H], F32)
    wk_my = din("wk_my", [D, H * PH])
    wk_ot = din("wk_ot", [D, H * PH])
    bk_my = din("bk_my", [128, H], F32)
    bk_ot = din("bk_ot", [128, H], F32)
    wv_my = din("wv_my", [D, VW], BF16)
    wv_ot = din("wv_ot", [D, VW], BF16)
    bv_my = din("bv_my", [1, VW], F32)
    bv_ot = din("bv_ot", [1, VW], F32)
    cos_t = din("cos_t", [128, N], BF16)
    sin_t = din("sin_t", [128, N], BF16)
    qk_s2 = din("qk_s2", [128, 1], F32)
    wo = din("wo", [96, H * D], BF16)
    ob_g = din("ob_g", [1, D], F32)
    w1 = din("w1", [D, MLPD], BF16)
    b1c = din("b1c", [128, MT2], F32)
    w2 = din("w2", [MLPD, D], BF16)
    b2r = din("b2r", [1, D], F32)

    my_out = nc.dram_tensor("my_out", [CH, D], F32, kind="ExternalOutput").ap()

    with tile.TileContext(nc) as tc:
        persist_cm = tc.tile_pool(name="persist", bufs=1)
        pp = persist_cm.__enter__()

        ident = pp.tile([128, 128], F32, name="ident")
        make_identity(nc, ident[:])
        mod_l2 = pp.tile([128, 36], F32, name="mod_l2")
        g_my_bc = pp.tile([128, D], F32, name="g_my_bc")
        m3g_bc = pp.tile([128, D], F32, name="m3g_bc")
        ob_bc = pp.tile([128, D], F32, name="ob_bc")
        b2_bc = pp.tile([128, D], F32, name="b2_bc")
        w1p = pp.tile([128, KT], F32, name="w1p")
        w2p = pp.tile([128, KT], F32, name="w2p")
        w3p = pp.tile([128, KT], F32, name="w3p")
        s2_sb = pp.tile([128, 1], F32, name="s2_sb")
        bq_sb = pp.tile([128, H], F32, name="bq_sb")
        bkm_sb = pp.tile([128, H], F32, name="bkm_sb")
        bko_sb = pp.tile([128, H], F32, name="bko_sb")
        eps_sb = pp.tile([128, 1], F32, name="eps_sb")
        nc.vector.memset(eps_sb[:], EPS)

        # ---------------- modulation MLP ----------------
        with (
            nc.named_scope("mod"),
            tc.tile_pool(name="modw", bufs=1) as mw,
            tc.tile_pool(name="psm", bufs=1, space="PSUM") as psm,
            tc.tile_pool(name="psg", bufs=2, space="PSUM") as psg,
        ):
            p_sb = mw.tile([128, 8], BF16, name="p_sb")
            nc.sync.dma_start(out=p_sb[:], in_=p_my.rearrange("o (j r) -> r (o j)", r=128))
            ps2 = mw.tile([128, 8], BF16, name="ps2")
            nc.scalar.activation(ps2[:], p_sb[:], AF.Silu)

            w1m_sb = mw.tile([128, 8, 512], BF16, name="w1m_sb")
            nc.sync.dma_start(out=w1m_sb[:], in_=mod_w1.rearrange("(k p) m -> p k m", p=128))
            b1m_sb = mw.tile([128, 4], F32, name="b1m_sb")
            nc.sync.dma_start(out=b1m_sb[:], in_=mod_b1)
            h_ps = psm.tile([128, 4], F32, name="h_ps")
            for mt in range(4):
                for kt in range(8):
                    nc.tensor.matmul(
                        h_ps[:, mt : mt + 1],
                        w1m_sb[:, kt, mt * 128 : (mt + 1) * 128],
                        ps2[:, kt : kt + 1],
                        start=(kt == 0), stop=(kt == 7),
                    )
            h_l2 = mw.tile([128, 4], BF16, name="h_l2")
            for mt in range(4):
                nc.scalar.activation(h_l2[:, mt : mt + 1], h_ps[:, mt : mt + 1],
                                     AF.Silu, bias=b1m_sb[:, mt : mt + 1])

            w2m_sb = mw.tile([128, 4, 6 * D], BF16, name="w2m_sb")
            nc.sync.dma_start(out=w2m_sb[:], in_=mod_w2m.rearrange("(k p) m -> p k m", p=128))
            b2m_sb = mw.tile([128, 36], F32, name="b2m_sb")
            nc.sync.dma_start(out=b2m_sb[:], in_=mod_b2m)
            mod_ps = psm.tile([128, 36], F32, name="mod_ps")
            for mt in range(36):
                for kt in range(4):
                    nc.tensor.matmul(
                        mod_ps[:, mt : mt + 1],
                        w2m_sb[:, kt, mt * 128 : (mt + 1) * 128],
                        h_l2[:, kt : kt + 1],
                        start=(kt == 0), stop=(kt == 3),
                    )
            nc.vector.tensor_add(mod_l2[:], mod_ps[:], b2m_sb[:])

            w2g_sb = mw.tile([128, 4, 2 * D], BF16, name="w2g_sb")
            nc.sync.dma_start(out=w2g_sb[:], in_=mod_w2g.rearrange("(k p) m -> p k m", p=128))
            b2g_sb = mw.tile([1, 2 * D], F32, name="b2g_sb")
            nc.sync.dma_start(out=b2g_sb[:], in_=mod_b2g)
            gates = mw.tile([1, 2 * D], F32, name="gates")
            for nt in range(3):
                g_ps = psg.tile([1, 512], F32, name="g_ps", tag="g_ps")
                for kt in range(4):
                    nc.tensor.matmul(
                        g_ps[:], h_l2[:, kt : kt + 1],
                        w2g_sb[:, kt, nt * 512 : (nt + 1) * 512],
                        start=(kt == 0), stop=(kt == 3),
                    )
                nc.vector.tensor_tensor(gates[:, nt * 512 : (nt + 1) * 512], g_ps[:],
                                        b2g_sb[:, nt * 512 : (nt + 1) * 512], op=ALU.add)
            nc.gpsimd.partition_broadcast(g_my_bc[:], gates[:, 0:D])
            nc.gpsimd.partition_broadcast(m3g_bc[:], gates[:, D : 2 * D])

            obg_sb = mw.tile([1, D], F32, name="obg_sb")
            nc.sync.dma_start(out=obg_sb[:], in_=ob_g)
            nc.gpsimd.partition_broadcast(ob_bc[:], obg_sb[:])
            b2r_sb = mw.tile([1, D], F32, name="b2r_sb")
            nc.sync.dma_start(out=b2r_sb[:], in_=b2r)
            nc.gpsimd.partition_broadcast(b2_bc[:], b2r_sb[:])

            n1my_sb = mw.tile([128, KT], F32, name="n1my_sb")
            n1ot_sb = mw.tile([128, KT], F32, name="n1ot_sb")
            n2my_sb = mw.tile([128, KT], F32, name="n2my_sb")
            nc.sync.dma_start(out=n1my_sb[:], in_=norm1_my)
            nc.sync.dma_start(out=n1ot_sb[:], in_=norm1_ot)
            nc.sync.dma_start(out=n2my_sb[:], in_=norm2_my)
            tmp6 = mw.tile([128, KT], F32, name="tmp6")
            nc.vector.tensor_scalar_add(tmp6[:], mod_l2[:, 0:6], 1.0)
            nc.vector.tensor_mul(w1p[:], n1my_sb[:], tmp6[:])
            tmp6b = mw.tile([128, KT], F32, name="tmp6b")
            nc.vector.tensor_scalar_add(tmp6b[:], mod_l2[:, 12:18], 1.0)
            nc.vector.tensor_mul(w2p[:], n1ot_sb[:], tmp6b[:])
            tmp6c = mw.tile([128, KT], F32, name="tmp6c")
            nc.vector.tensor_scalar_add(tmp6c[:], mod_l2[:, 24:30], 1.0)
            nc.vector.tensor_mul(w3p[:], n2my_sb[:], tmp6c[:])
            nc.sync.dma_start(out=s2_sb[:], in_=qk_s2)
            nc.sync.dma_start(out=bq_sb[:], in_=bq)
            nc.sync.dma_start(out=bkm_sb[:], in_=bk_my)
            nc.sync.dma_start(out=bko_sb[:], in_=bk_ot)

        # ---------------- big persistent activations ----------------
        x1n = pp.tile([128, 4, D], F32R, name="x1n")
        with tc.tile_pool(name="poolA", bufs=1) as pa:
            K_sb = pa.tile([128, H, N], BF16, name="K_sb")
            V_sb = pa.tile([128, N // 128, VW], BF16, name="V_sb")
            Q_sb = pa.tile([128, H, CH], BF16, name="Q_sb")
            cos_sb = pa.tile([128, N], BF16, name="cos_sb")
            sin_sb = pa.tile([128, N], BF16, name="sin_sb")
            nc.sync.dma_start(out=cos_sb[:], in_=cos_t)
            nc.sync.dma_start(out=sin_sb[:], in_=sin_t)

            # ---------------- phase 1: xm + Q/K/V projections + rope ----------------
            with (
                nc.named_scope("proj"),
                tc.tile_pool(name="wkvp", bufs=1) as wkvp,
                tc.tile_pool(name="ph1", bufs=1) as ph1,
                tc.tile_pool(name="ph1b", bufs=2) as ph1b,
                tc.tile_pool(name="psP", bufs=2, space="PSUM") as psP,
                tc.tile_pool(name="psV", bufs=2, space="PSUM") as psV,
                tc.tile_pool(name="psT", bufs=2, space="PSUM") as psT,
            ):
                wk_cur = None
                wv_cur = None
                bv_cur = None
                for sl in range(4):
                    my_stream = sl < 2
                    x_l1 = ph1b.tile([128, 4, D], F32, name="x_l1", tag="x_l1")
                    src = x_own if sl == 0 else x_rest[sl - 1]
                    nc.sync.dma_start(out=x_l1[:], in_=src.rearrange("(t p) c -> p t c", p=128))

                    # rms: xs = x * rstd, in place (stats batched over the 4 tok-tiles)
                    ssq4 = ph1.tile([128, 4], F32, name="ssq4b", tag="ssq4b")
                    for tt in range(4):
                        sq = ph1.tile([128, D], F32, name="sq", tag="sq")
                        nc.scalar.activation(sq[:], x_l1[:, tt, :], AF.Square,
                                             accum_out=ssq4[:, tt : tt + 1])
                    rstd4 = ph1.tile([128, 4], F32, name="rstd4b", tag="rstd4b")
                    nc.scalar.activation(rstd4[:], ssq4[:], AF.Abs_reciprocal_sqrt,
                                         scale=1.0 / D, bias=eps_sb[:])
                    for tt in range(4):
                        nc.vector.tensor_scalar_mul(x_l1[:, tt, :], x_l1[:, tt, :],
                                                    rstd4[:, tt : tt + 1])

                    # transpose + modulate -> xm_l2 (f32r) and a bf16 copy for V
                    xm_l2 = ph1b.tile([128, KT, CH], F32R, name="xm_l2", tag="xm_l2")
                    wsel = w1p if my_stream else w2p
                    hoff = 6 if my_stream else 18
                    for tt in range(4):
                        for ft in range(KT):
                            tp = psT.tile([128, 128], F32, name="tp", tag="tp")
                            nc.tensor.transpose(tp[:], x_l1[:, tt, ft * 128 : (ft + 1) * 128], ident[:])
                            nc.vector.tensor_scalar(
                                xm_l2[:, ft, tt * 128 : (tt + 1) * 128], tp[:],
                                wsel[:, ft : ft + 1], mod_l2[:, hoff + ft : hoff + ft + 1],
                                op0=ALU.mult, op1=ALU.add,
                            )
                    xm_bf = ph1.tile([128, KT, CH], BF16, name="xm_bf", tag="xm_bf")
                    nc.vector.tensor_copy(xm_bf[:], xm_l2[:])

                    # Q projection (own chunk only)
                    if sl == 0:
                        wq_sb = wkvp.tile([128, KT, H * PH], F32R, name="wq_sb", tag="wbig")
                        nc.sync.dma_start(out=wq_sb[:], in_=wq.rearrange("(k p) m -> p k m", p=128))
                        for h in range(H):
                            qp = psP.tile([128, CH], F32, name="qp", tag="qp")
                            for kt in range(KT):
                                nc.tensor.matmul(
                                    qp[:], wq_sb[:, kt, h * PH : (h + 1) * PH],
                                    xm_l2[:, kt, :], start=(kt == 0), stop=(kt == KT - 1),
                                )
                            nc.scalar.activation(Q_sb[:, h, :], qp[:], AF.Identity,
                                                 bias=bq_sb[:, h : h + 1])

                    # K projection
                    if sl in (0, 2):
                        wk_sb = wkvp.tile([128, KT, H * PH], F32R, name="wk_sb", tag="wbig")
                        nc.sync.dma_start(
                            out=wk_sb[:],
                            in_=(wk_my if my_stream else wk_ot).rearrange("(k p) m -> p k m", p=128),
                        )
                        wk_cur = wk_sb
                    bsel = bkm_sb if my_stream else bko_sb
                    for h in range(H):
                        kp = psP.tile([128, CH], F32, name="kp", tag="qp")
                        for kt in range(KT):
                            nc.tensor.matmul(
                                kp[:], wk_cur[:, kt, h * PH : (h + 1) * PH],
                                xm_l2[:, kt, :], start=(kt == 0), stop=(kt == KT - 1),
                            )
                        nc.scalar.activation(K_sb[:, h, sl * CH : (sl + 1) * CH], kp[:],
                                             AF.Identity, bias=bsel[:, h : h + 1])

                    # V projection, direct L1
                    if sl in (0, 2):
                        wv_sb = wkvp.tile([128, KT, VW], BF16, name="wv_sb", tag="wv")
                        nc.sync.dma_start(
                            out=wv_sb[:],
                            in_=(wv_my if my_stream else wv_ot).rearrange("(k p) m -> p k m", p=128),
                        )
                        bv_bc = wkvp.tile([128, VW], F32, name="bv_bc", tag="bv_bc")
                        bv_row = ph1.tile([1, VW], F32, name="bv_row", tag="bv_row")
                        nc.sync.dma_start(out=bv_row[:], in_=(bv_my if my_stream else bv_ot))
                        nc.gpsimd.partition_broadcast(bv_bc[:], bv_row[:])
                        wv_cur = wv_sb
                        bv_cur = bv_bc
                    for tt in range(4):
                        vp1 = psV.tile([128, 512], F32, name="vp1", tag="vp1")
                        vp2 = psV.tile([128, VW - 512], F32, name="vp2", tag="vp2")
                        for kt in range(KT):
                            nc.tensor.matmul(
                                vp1[:], xm_bf[:, kt, tt * 128 : (tt + 1) * 128],
                                wv_cur[:, kt, 0:512], start=(kt == 0), stop=(kt == KT - 1),
                            )
                        for kt in range(KT):
                            nc.tensor.matmul(
                                vp2[:], xm_bf[:, kt, tt * 128 : (tt + 1) * 128],
                                wv_cur[:, kt, 512:VW], start=(kt == 0), stop=(kt == KT - 1),
                            )
                        nc.vector.tensor_tensor(V_sb[:, sl * 4 + tt, 0:512], vp1[:],
                                                bv_cur[:, 0:512], op=ALU.add)
                        nc.vector.tensor_tensor(V_sb[:, sl * 4 + tt, 512:VW], vp2[:],
                                                bv_cur[:, 512:VW], op=ALU.add)

                    # rope on this K chunk (half the heads at a time; +-64 shifted writes)
                    c3 = cos_sb[:, sl * CH : (sl + 1) * CH]
                    s3 = sin_sb[:, sl * CH : (sl + 1) * CH]
                    HG = H // 2
                    for hg in range(2):
                        kr_t = ph1.tile([128, HG, CH], BF16, name="kr_t", tag="kr_t")
                        kr_m = ph1.tile([128, HG, CH], BF16, name="kr_m", tag="kr_m")
                        ksl = K_sb[:, hg * HG : (hg + 1) * HG, sl * CH : (sl + 1) * CH]
                        nc.vector.tensor_tensor(kr_t[:], ksl, _bc3(c3, HG), op=ALU.mult)
                        nc.vector.tensor_tensor(kr_m[0:64], ksl[64:128], _bc3(s3[64:128], HG), op=ALU.mult)
                        nc.vector.tensor_tensor(kr_m[64:128], ksl[0:64], _bc3(s3[0:64], HG), op=ALU.mult)
                        nc.vector.tensor_tensor(ksl[0:64], kr_t[0:64], kr_m[0:64], op=ALU.subtract)
                        nc.vector.tensor_tensor(ksl[64:128], kr_t[64:128], kr_m[64:128], op=ALU.add)

                    if sl == 0:
                        c0 = cos_sb[:, 0:CH]
                        s0 = sin_sb[:, 0:CH]
                        for hg in range(2):
                            qr_t = ph1.tile([128, HG, CH], BF16, name="qr_t", tag="kr_t")
                            qr_m = ph1.tile([128, HG, CH], BF16, name="qr_m", tag="kr_m")
                            qsl = Q_sb[:, hg * HG : (hg + 1) * HG, :]
                            nc.vector.tensor_tensor(qr_t[:], qsl, _bc3(c0, HG), op=ALU.mult)
                            nc.vector.tensor_tensor(qr_m[0:64], qsl[64:128], _bc3(s0[64:128], HG), op=ALU.mult)
                            nc.vector.tensor_tensor(qr_m[64:128], qsl[0:64], _bc3(s0[0:64], HG), op=ALU.mult)
                            nc.vector.tensor_tensor(qsl[0:64], qr_t[0:64], qr_m[0:64], op=ALU.subtract)
                            nc.vector.tensor_tensor(qsl[64:128], qr_t[64:128], qr_m[64:128], op=ALU.add)

            # ---------------- phases 2+3: qk-norm, attention, out-proj, residual ----------------
            with (
                tc.tile_pool(name="ph2", bufs=2) as ph2,
                tc.tile_pool(name="ph2s", bufs=1) as ph2s,
                tc.tile_pool(name="ph3w", bufs=1) as ph3w,
            ):
                attnn = ph2s.tile([96, H, CH], BF16, name="attnn")
                with (
                    nc.named_scope("attn"),
                    tc.tile_pool(name="psK", bufs=2, space="PSUM") as psK,
                                        tc.tile_pool(name="psS", bufs=2, space="PSUM") as psS,
                    tc.tile_pool(name="psPV", bufs=2, space="PSUM") as psPV,
                ):
                    ones_bf = ph2s.tile([128, 1], BF16, name="ones_bf")
                    nc.vector.memset(ones_bf[:], 1.0)
                    ones = ph2s.tile([128, 1], F32, name="ones")
                    nc.vector.memset(ones[:], 1.0)

                    # rk_all[kt-token, h*16+kt2] = 1/(sqrt(dh)*|k|), per-partition layout
                    rk_all = ph2s.tile([128, H * 16], F32, name="rk_all")
                    rk_ps = psK.tile([128, H * 16], F32, name="rk_ps", tag="rk_ps")
                    for h in range(H):
                        ksq = ph2.tile([128, N], BF16, name="ksq", tag="ksq")
                        nc.vector.tensor_mul(ksq[:], K_sb[:, h, :], K_sb[:, h, :])
                        for kt2 in range(16):
                            nc.tensor.matmul(
                                rk_ps[:, h * 16 + kt2 : h * 16 + kt2 + 1],
                                ksq[:, kt2 * 128 : (kt2 + 1) * 128],
                                ones_bf[:], start=True, stop=True,
                            )
                    nc.scalar.activation(rk_all[:], rk_ps[:], AF.Abs_reciprocal_sqrt,
                                         scale=float(DH), bias=eps_sb[:])

                    # q_hat = q * s2 * (1/|q|)
                    for h in range(H):
                        qsq = ph2.tile([128, CH], BF16, name="qsq", tag="qsq")
                        nc.vector.tensor_mul(qsq[:], Q_sb[:, h, :], Q_sb[:, h, :])
                        rq_ps = psK.tile([1, CH], F32, name="rq_ps", tag="rq_ps")
                        nc.tensor.matmul(rq_ps[:], ones_bf[:], qsq[:], start=True, stop=True)
                        rq_bf = ph2.tile([1, CH], BF16, name="rq_bf", tag="rq_bf")
                        nc.scalar.activation(rq_bf[:], rq_ps[:], AF.Abs_reciprocal_sqrt,
                                             bias=eps_sb[0:1, :])
                        rq_bc = ph2.tile([128, CH], BF16, name="rq_bc", tag="rq_bc")
                        nc.gpsimd.partition_broadcast(rq_bc[:], rq_bf[:])
                        nc.vector.scalar_tensor_tensor(
                            Q_sb[:, h, :], Q_sb[:, h, :], s2_sb[:], rq_bc[:],
                            op0=ALU.mult, op1=ALU.mult,
                        )

                    for h in range(H):
                        pv = psPV.tile([128, CH], F32, name="pv", tag="pv")
                        for kt2 in range(16):
                            sps = psS.tile([128, CH], F32, name="sps", tag="sps")
                            nc.tensor.matmul(
                                sps[:], K_sb[:, h, kt2 * 128 : (kt2 + 1) * 128],
                                Q_sb[:, h, :], start=True, stop=True,
                            )
                            pt = ph2.tile([128, CH], BF16, name="pt", tag="pt")
                            nc.scalar.activation(pt[:], sps[:], AF.Exp,
                                                 scale=rk_all[:, h * 16 + kt2 : h * 16 + kt2 + 1])
                            nc.tensor.matmul(
                                pv[0:97, :], V_sb[:, kt2, h * 97 : (h + 1) * 97],
                                pt[:], start=(kt2 == 0), stop=(kt2 == 15),
                            )
                        rs2 = ph2.tile([1, CH], F32, name="rs2", tag="rs2")
                        nc.scalar.activation(rs2[:], pv[96:97, :], AF.Square)
                        rs = ph2.tile([1, CH], F32, name="rs", tag="rs")
                        nc.scalar.activation(rs[:], rs2[:], AF.Abs_reciprocal_sqrt)
                        rs_bc = ph2.tile([96, CH], F32, name="rs_bc", tag="rs_bc")
                        nc.gpsimd.partition_broadcast(rs_bc[:], rs[:], channels=96)
                        nc.vector.tensor_tensor(attnn[:, h, :], pv[0:96, :], rs_bc[:], op=ALU.mult)

                # out-proj + residual
                with (
                    nc.named_scope("oproj"),
                    tc.tile_pool(name="psO", bufs=2, space="PSUM") as psO,
                ):
                    wo_sb = ph3w.tile([96, H, D], BF16, name="wo_sb")
                    nc.sync.dma_start(out=wo_sb[:], in_=wo.rearrange("p (h m) -> p h m", h=H))
                    xo_l1 = ph3w.tile([128, 4, D], F32, name="xo_l1")
                    nc.sync.dma_start(out=xo_l1[:], in_=x_own2.rearrange("(t p) c -> p t c", p=128))
                    for qt in range(4):
                        op1 = psO.tile([128, 512], F32, name="op1", tag="op1")
                        op2 = psO.tile([128, D - 512], F32, name="op2", tag="op2")
                        for h in range(H):
                            nc.tensor.matmul(
                                op1[:], attnn[:, h, qt * 128 : (qt + 1) * 128],
                                wo_sb[:, h, 0:512], start=(h == 0), stop=(h == H - 1),
                            )
                        for h in range(H):
                            nc.tensor.matmul(
                                op2[:], attnn[:, h, qt * 128 : (qt + 1) * 128],
                                wo_sb[:, h, 512:D], start=(h == 0), stop=(h == H - 1),
                            )
                        t1 = ph2.tile([128, D], F32, name="t1", tag="t1")
                        nc.vector.tensor_tensor(t1[:, 0:512], op1[:], ob_bc[:, 0:512], op=ALU.add)
                        nc.vector.tensor_tensor(t1[:, 512:D], op2[:], ob_bc[:, 512:D], op=ALU.add)
                        nc.vector.tensor_mul(t1[:], t1[:], g_my_bc[:])
                        nc.vector.tensor_tensor(x1n[:, qt, :], t1[:], xo_l1[:, qt, :], op=ALU.add)

        # ---------------- phase 4: norm2 + MLP + final ----------------
        with (
            nc.named_scope("mlp"),
            tc.tile_pool(name="ph4", bufs=2) as ph4,
            tc.tile_pool(name="mlpw", bufs=1) as mlpw,
            tc.tile_pool(name="psM", bufs=2, space="PSUM") as psM,
            tc.tile_pool(name="psM2", bufs=2, space="PSUM") as psM2,
            tc.tile_pool(name="psT2", bufs=2, space="PSUM") as psT2,
        ):
            w1_sb = mlpw.tile([128, KT, MLPD], BF16, name="w1_sb")
            nc.sync.dma_start(out=w1_sb[:], in_=w1.rearrange("(k p) m -> p k m", p=128))
            w2_sb = mlpw.tile([128, MT2, D], BF16, name="w2_sb")
            nc.sync.dma_start(out=w2_sb[:], in_=w2.rearrange("(k p) m -> p k m", p=128))
            b1_sb = mlpw.tile([128, MT2], F32, name="b1_sb")
            nc.sync.dma_start(out=b1_sb[:], in_=b1c)

            xn_l2 = mlpw.tile([128, KT, CH], BF16, name="xn_l2")
            ssq4m = ph4.tile([128, 4], F32, name="ssq4m", tag="ssq4m")
            for tt in range(4):
                sq = ph4.tile([128, D], F32, name="sq4", tag="sq4")
                nc.scalar.activation(sq[:], x1n[:, tt, :], AF.Square,
                                     accum_out=ssq4m[:, tt : tt + 1])
            rstd4m = ph4.tile([128, 4], F32, name="rstd4m", tag="rstd4m")
            nc.scalar.activation(rstd4m[:], ssq4m[:], AF.Abs_reciprocal_sqrt,
                                 scale=1.0 / D, bias=eps_sb[:])
            for tt in range(4):
                xs = ph4.tile([128, D], F32, name="xs4", tag="xs4")
                nc.vector.tensor_scalar_mul(xs[:], x1n[:, tt, :], rstd4m[:, tt : tt + 1])
                for ft in range(KT):
                    tp = psT2.tile([128, 128], F32, name="tp2", tag="tp2")
                    nc.tensor.transpose(tp[:], xs[:, ft * 128 : (ft + 1) * 128], ident[:])
                    nc.vector.tensor_scalar(
                        xn_l2[:, ft, tt * 128 : (tt + 1) * 128], tp[:],
                        w3p[:, ft : ft + 1], mod_l2[:, 30 + ft : 30 + ft + 1],
                        op0=ALU.mult, op1=ALU.add,
                    )

            h_bf = mlpw.tile([128, MT2, CH], BF16, name="h_bf")
            for mt in range(MT2):
                fp = psM.tile([128, CH], F32, name="fp", tag="fp")
                for kt in range(KT):
                    nc.tensor.matmul(
                        fp[:], w1_sb[:, kt, mt * 128 : (mt + 1) * 128],
                        xn_l2[:, kt, :], start=(kt == 0), stop=(kt == KT - 1),
                    )
                nc.scalar.activation(h_bf[:, mt, :], fp[:], AF.Gelu,
                                     bias=b1_sb[:, mt : mt + 1])

            out_f = mlpw.tile([128, 4, D], F32, name="out_f")
            for qt in range(4):
                f1 = psM2.tile([128, 512], F32, name="f1", tag="f1")
                f2 = psM2.tile([128, D - 512], F32, name="f2", tag="f2")
                for mt in range(MT2):
                    nc.tensor.matmul(
                        f1[:], h_bf[:, mt, qt * 128 : (qt + 1) * 128],
                        w2_sb[:, mt, 0:512], start=(mt == 0), stop=(mt == MT2 - 1),
                    )
                for mt in range(MT2):
                    nc.tensor.matmul(
                        f2[:], h_bf[:, mt, qt * 128 : (qt + 1) * 128],
                        w2_sb[:, mt, 512:D], start=(mt == 0), stop=(mt == MT2 - 1),
                    )
                t2 = ph4.tile([128, D], F32, name="t2", tag="t2")
                nc.vector.tensor_tensor(t2[:, 0:512], f1[:], b2_bc[:, 0:512], op=ALU.add)
                nc.vector.tensor_tensor(t2[:, 512:D], f2[:], b2_bc[:, 512:D], op=ALU.add)
                nc.vector.tensor_mul(t2[:], t2[:], m3g_bc[:])
                nc.vector.tensor_tensor(out_f[:, qt, :], t2[:], x1n[:, qt, :], op=ALU.add)
            nc.sync.dma_start(out=my_out.rearrange("(t p) c -> p t c", p=128), in_=out_f[:])

        persist_cm.__exit__(None, None, None)


    nc.compile()
    _CACHED["nc"] = nc
    return nc


def _pad_head_cols(w_h, b_h):
    wp = np.zeros((D, PH), np.float32)
    bp = np.zeros((PH,), np.float32)
    wp[:, _ROWS_LO] = w_h[:, 0:48]
    wp[:, _ROWS_HI] = w_h[:, 48:96]
    bp[_ROWS_LO] = b_h[0:48]
    bp[_ROWS_HI] = b_h[48:96]
    return wp, bp


def _prep_core_inputs(c, inp):
    b, r = c // 4, c % 4
    s = 0 if r < 2 else 1
    sub = r % 2

    x1 = np.asarray(inp["x_stream1"], np.float32)
    x2 = np.asarray(inp["x_stream2"], np.float32)
    xs_ = [x1[b], x2[b]]
    my, ot = xs_[s], xs_[1 - s]
    x_own = np.ascontiguousarray(my[sub * CH : (sub + 1) * CH])
    x_rest = np.ascontiguousarray(np.stack([
        my[(1 - sub) * CH : (2 - sub) * CH],
        ot[0:CH],
        ot[CH : 2 * CH],
    ]))

    pos = np.concatenate([
        np.arange(s * T + sub * CH, s * T + (sub + 1) * CH),
        np.arange(s * T + (1 - sub) * CH, s * T + (2 - sub) * CH),
        np.arange((1 - s) * T, (1 - s) * T + CH),
        np.arange((1 - s) * T + CH, (1 - s) * T + 2 * CH),
    ])
    inv = (1.0 / (10000.0 ** (np.arange(0, DH, 2, dtype=np.float32) / DH)))
    inv = inv.astype(_BF16).astype(np.float32)
    freqs = pos[:, None].astype(np.float32) * inv[None, :]
    emb = np.concatenate([freqs, freqs], axis=-1)
    cos_d, sin_d = np.cos(emb), np.sin(emb)
    cos_p = np.zeros((128, N), np.float32)
    sin_p = np.zeros((128, N), np.float32)
    cos_p[_ROWS_LO] = cos_d[:, 0:48].T
    cos_p[_ROWS_HI] = cos_d[:, 48:96].T
    sin_p[_ROWS_LO] = sin_d[:, 48:96].T
    sin_p[_ROWS_HI] = sin_d[:, 0:48].T

    qkv_w = [np.asarray(inp["qkv_w"], np.float32), np.asarray(inp["qkv2_w"], np.float32)]
    qkv_b = [np.asarray(inp["qkv_b"], np.float32), np.asarray(inp["qkv2_b"], np.float32)]

    def qkv_part(si, part):
        return qkv_w[si][:, part * D : (part + 1) * D], qkv_b[si][part * D : (part + 1) * D]

    def padded(si, part):
        wfull, bfull = qkv_part(si, part)
        wp = np.zeros((D, H * PH), np.float32)
        bp = np.zeros((128, H), np.float32)
        for h in range(H):
            whp, bhp = _pad_head_cols(wfull[:, h * DH : (h + 1) * DH],
                                      bfull[h * DH : (h + 1) * DH])
            wp[:, h * PH : (h + 1) * PH] = whp
            bp[:, h] = bhp
        return wp, bp

    wq_p, bq_p = padded(s, 0)
    wkm_p, bkm_p = padded(s, 1)
    wko_p, bko_p = padded(1 - s, 1)

    def v_aug(si):
        wfull, bfull = qkv_part(si, 2)
        wa = np.zeros((D, VW), np.float32)
        ba = np.zeros((1, VW), np.float32)
        for h in range(H):
            wa[:, h * 97 : h * 97 + 96] = wfull[:, h * DH : (h + 1) * DH]
            ba[0, h * 97 : h * 97 + 96] = bfull[h * DH : (h + 1) * DH]
            ba[0, h * 97 + 96] = 1.0
        return wa.astype(_BF16), ba

    wvm_a, bvm_a = v_aug(s)
    wvo_a, bvo_a = v_aug(1 - s)

    qs = np.asarray(inp["qk_scale"], np.float32)
    s2 = np.zeros((128, 1), np.float32)
    s2[_ROWS_LO, 0] = qs[0:48] ** 2
    s2[_ROWS_HI, 0] = qs[48:96] ** 2

    def l2cols(v):
        return np.ascontiguousarray(np.asarray(v, np.float32).reshape(KT, 128).T)

    ms_my, mh_my, g_my = (0, 1, 2) if s == 0 else (3, 4, 5)
    ms_ot, mh_ot = (3, 4) if s == 0 else (0, 1)
    m3s, m3h, m3g = (6, 7, 8) if s == 0 else (9, 10, 11)

    w2f = np.asarray(inp["mod_w2"], np.float32)
    b2f = np.asarray(inp["mod_b2"], np.float32)
    cw = lambda i: w2f[:, i * D : (i + 1) * D]
    cb = lambda i: b2f[i * D : (i + 1) * D]
    main_idx = [ms_my, mh_my, ms_ot, mh_ot, m3s, m3h]
    mod_w2m = np.concatenate([cw(i) for i in main_idx], axis=1).astype(_BF16)
    mod_b2m = np.ascontiguousarray(np.concatenate([l2cols(cb(i)) for i in main_idx], axis=1))
    mod_w2g = np.concatenate([cw(g_my), cw(m3g)], axis=1).astype(_BF16)
    mod_b2g = np.ascontiguousarray(np.concatenate([cb(g_my), cb(m3g)])[None, :])

    wo_f = np.asarray(inp["out_w"], np.float32)
    wo_dev = np.ascontiguousarray(wo_f.reshape(H, DH, D).transpose(1, 0, 2).reshape(DH, H * D))

    norm1 = [np.asarray(inp["norm11_w"], np.float32), np.asarray(inp["norm12_w"], np.float32)]
    norm2 = [np.asarray(inp["norm21_w"], np.float32), np.asarray(inp["norm22_w"], np.float32)]
    mlw = [
        (inp["mlp1_w1"], inp["mlp1_b1"], inp["mlp1_w2"], inp["mlp1_b2"]),
        (inp["mlp2_w1"], inp["mlp2_b1"], inp["mlp2_w2"], inp["mlp2_b2"]),
    ]
    w1f, b1f, w2mf, b2mf = [np.asarray(a, np.float32) for a in mlw[s]]

    return {
        "x_own": x_own,
        "x_rest": x_rest,
        "x_own2": x_own.copy(),
        "p_my": np.asarray(inp["p_emb"], np.float32)[b].astype(_BF16),
        "mod_w1": np.asarray(inp["mod_w1"], np.float32).astype(_BF16),
        "mod_b1": np.ascontiguousarray(np.asarray(inp["mod_b1"], np.float32).reshape(4, 128).T),
        "mod_w2m": mod_w2m,
        "mod_b2m": mod_b2m,
        "mod_w2g": mod_w2g,
        "mod_b2g": mod_b2g,
        "norm1_my": l2cols(norm1[s]),
        "norm1_ot": l2cols(norm1[1 - s]),
        "norm2_my": l2cols(norm2[s]),
        "wq": wq_p, "bq": bq_p,
        "wk_my": wkm_p, "bk_my": bkm_p,
        "wk_ot": wko_p, "bk_ot": bko_p,
        "wv_my": wvm_a, "bv_my": bvm_a,
        "wv_ot": wvo_a, "bv_ot": bvo_a,
        "cos_t": cos_p.astype(_BF16), "sin_t": sin_p.astype(_BF16), "qk_s2": s2,
        "wo": wo_dev.astype(_BF16),
        "ob_g": np.ascontiguousarray(np.asarray(inp["out_b"], np.float32)[None, :]),
        "w1": w1f.astype(_BF16),
        "b1c": np.ascontiguousarray(b1f.reshape(MT2, 128).T),
        "w2": w2mf.astype(_BF16),
        "b2r": np.ascontiguousarray(b2mf[None, :]),
    }


def kernel(**inputs):
    nc = _build()
    in_maps = [_prep_core_inputs(c, inputs) for c in range(NC)]
    res = run_bass_kernel_spmd(nc, in_maps, core_ids=list(range(NC)), trace=False)
    out1 = np.zeros((B, T, D), np.float32)
    out2 = np.zeros((B, T, D), np.float32)
    for c in range(NC):
        b, r = c // 4, c % 4
        dst = out1 if r < 2 else out2
        sub = r % 2
        dst[b, sub * CH : (sub + 1) * CH] = res.results[c]["my_out"]
    return out1, out2



# revision 18
# speedup vs baseline: 1.1848x; 1.1848x over previous
"""Trainium2 Bass kernel for nn_DoubleStream_Expert (dense double-stream DiT block).

Sharding (8 cores, AllGather K/V): core c -> batch b = c//4, group rank r = c%4.
Each core computes Q/K/V projections, rope and qk-norm for ONLY its own 512
tokens (chunk r of the batch's concatenated 2048-token sequence), then the
4 cores of each batch AllGather K-hat/V via HBM collectives. Attention, the
out-projection and the MLP run on the core's own 512 queries.

qk-norm folding: 1/(sqrt(dh)|k|) is folded into K columns (K-hat) and
s^2/|q| into Q before the gather, so the softmax exp needs no per-block
scale and can run in large [128, 1024] batches on the Scalar engine.

The attention main loop skips the core's own chunk (already processed from
local tiles while the collective is in flight) by indexing the gathered
K/V with per-core slot registers (bass.ds) loaded from an int32 input --
the compiled program stays identical across cores.

Head dims are padded 96->128 with the rotary halves at rows 0..47 / 64..111;
V is augmented with a ones-column (97 per head) so the softmax denominator
falls out of the PV matmul. All matmuls bf16; stats/residuals fp32.
"""

import numpy as np

import concourse.bass as bass
import concourse.mybir as mybir
import concourse.tile as tile
from concourse import bacc
from concourse.bass_utils import run_bass_kernel_spmd
from concourse.masks import make_identity

try:
    import ml_dtypes
    _BF16 = ml_dtypes.bfloat16
except ImportError:  # pragma: no cover
    _BF16 = np.float32

F32 = mybir.dt.float32
BF16 = mybir.dt.bfloat16
I32 = mybir.dt.int32
AF = mybir.ActivationFunctionType
ALU = mybir.AluOpType

B, T, D, H, DH, MLPD = 2, 1024, 768, 8, 96, 3072
N = 2 * T
NC = 8
CH = 512
KT = D // 128         # 6
MT2 = MLPD // 128     # 24
PH = 128
VW = H * 97           # 776
KW = H * PH           # 1024
EPS = 1e-6
HG = H // 2

# True: attention main loop covers only the 3 remote chunks via dynamic slot
# registers; own chunk's probs are computed from local tiles during the AG.
DYN = True

_ROWS_LO = np.arange(0, 48)
_ROWS_HI = np.arange(64, 112)

_CACHED = {}


def _bc3(ap2d, nh):
    """[P, C] -> [P, nh, C] stride-0 broadcast over a middle axis."""
    return ap2d.unsqueeze(1).broadcast_to([ap2d.shape[0], nh, ap2d.shape[1]])


def _rope_inplace(nc, dst3, cos_ap, sin_ap, pool, nh, grp=2):
    """In-place rope on dst3 [128, nh, CH] (rows 0..63 lo / 64..127 hi)."""
    for hg in range(0, nh, grp):
        sl = dst3[:, hg : hg + grp, :]
        r_t = pool.tile([128, grp, CH], BF16, name="r_t", tag="r_t")
        r_m = pool.tile([128, grp, CH], BF16, name="r_m", tag="r_m")
        nc.vector.tensor_tensor(r_t[:], sl, _bc3(cos_ap, grp), op=ALU.mult)
        nc.vector.tensor_tensor(r_m[0:64], sl[64:128], _bc3(sin_ap[64:128], grp), op=ALU.mult)
        nc.vector.tensor_tensor(r_m[64:128], sl[0:64], _bc3(sin_ap[0:64], grp), op=ALU.mult)
        nc.vector.tensor_tensor(sl[0:64], r_t[0:64], r_m[0:64], op=ALU.subtract)
        nc.vector.tensor_tensor(sl[64:128], r_t[64:128], r_m[64:128], op=ALU.add)


def _build():
    if "nc" in _CACHED:
        return _CACHED["nc"]

    nc = bacc.Bacc("TRN2", target_bir_lowering=False, debug=False, num_devices=NC)

    def din(name, shape, dt=BF16):
        return nc.dram_tensor(name, list(shape), dt, kind="ExternalInput").ap()

    x_own = din("x_own", [CH, D], F32)
    p_my = din("p_my", [1, 1024], BF16)
    mod_w1 = din("mod_w1", [1024, 512], BF16)
    mod_b1 = din("mod_b1", [128, 4], F32)
    mod_w2m = din("mod_w2m", [512, 4 * D], BF16)   # ms_my mh_my m3s m3h
    mod_b2m = din("mod_b2m", [128, 24], F32)
    mod_w2g = din("mod_w2g", [512, 2 * D], BF16)   # g_my, m3g
    mod_b2g = din("mod_b2g", [1, 2 * D], F32)
    norm1_my = din("norm1_my", [128, KT], F32)
    norm2_my = din("norm2_my", [128, KT], F32)
    wq = din("wq", [D, KW])
    bq = din("bq", [128, H], F32)
    wk = din("wk", [D, KW])
    bk = din("bk", [128, H], F32)
    wv = din("wv", [D, VW])
    bv = din("bv", [1, VW], F32)
    cos_t = din("cos_t", [128, CH], BF16)
    sin_t = din("sin_t", [128, CH], BF16)
    qk_s2 = din("qk_s2", [128, 1], F32)
    wo = din("wo", [96, H * D], BF16)
    ob_g = din("ob_g", [1, D], F32)
    w1 = din("w1", [D, MLPD], BF16)
    b1c = din("b1c", [128, MT2], F32)
    w2 = din("w2", [MLPD, D], BF16)
    b2r = din("b2r", [1, D], F32)
    slots = din("slots", [1, 4], I32)

    my_out = nc.dram_tensor("my_out", [CH, D], F32, kind="ExternalOutput").ap()

    k_stage = nc.dram_tensor("k_stage", [128, H * CH], BF16, kind="Internal").ap()
    k_gath = nc.dram_tensor("k_gath", [4 * 128, H * CH], BF16, kind="Internal").ap()
    v_stage = nc.dram_tensor("v_stage", [128, 4 * VW], BF16, kind="Internal").ap()
    v_gath = nc.dram_tensor("v_gath", [4 * 128, 4 * VW], BF16, kind="Internal").ap()
    RG = [[0, 1, 2, 3], [4, 5, 6, 7]]

    with tile.TileContext(nc) as tc:
        persist_cm = tc.tile_pool(name="persist", bufs=1)
        pp = persist_cm.__enter__()

        ident = pp.tile([128, 128], F32, name="ident")
        make_identity(nc, ident[:])
        mod_l2 = pp.tile([128, 24], F32, name="mod_l2")
        g_my_bc = pp.tile([128, D], F32, name="g_my_bc")
        m3g_bc = pp.tile([128, D], F32, name="m3g_bc")
        ob_bc = pp.tile([128, D], F32, name="ob_bc")
        b2_bc = pp.tile([128, D], F32, name="b2_bc")
        w1p = pp.tile([128, KT], F32, name="w1p")
        w3p = pp.tile([128, KT], F32, name="w3p")
        s2_sb = pp.tile([128, 1], F32, name="s2_sb")
        bq_sb = pp.tile([128, H], F32, name="bq_sb")
        bk_sb = pp.tile([128, H], F32, name="bk_sb")
        eps_sb = pp.tile([128, 1], F32, name="eps_sb")
        nc.vector.memset(eps_sb[:], EPS)
        ones_bf = pp.tile([128, 1], BF16, name="ones_bf")
        nc.vector.memset(ones_bf[:], 1.0)
        slot_sb = pp.tile([1, 4], I32, name="slot_sb")
        nc.sync.dma_start(out=slot_sb[:], in_=slots)

        # x kept raw in SBUF for the attention residual (no second load)
        x_l1 = pp.tile([128, 4, D], F32, name="x_l1")
        nc.sync.dma_start(out=x_l1[:], in_=x_own.rearrange("(t p) c -> p t c", p=128))
        cos_sb = pp.tile([128, CH], BF16, name="cos_sb")
        sin_sb = pp.tile([128, CH], BF16, name="sin_sb")
        nc.sync.dma_start(out=cos_sb[:], in_=cos_t)
        nc.sync.dma_start(out=sin_sb[:], in_=sin_t)

        # ---------------- modulation MLP ----------------
        with (
            nc.named_scope("mod"),
            tc.tile_pool(name="modw", bufs=1) as mw,
            tc.tile_pool(name="psm", bufs=1, space="PSUM") as psm,
            tc.tile_pool(name="psg", bufs=2, space="PSUM") as psg,
        ):
            p_sb = mw.tile([128, 8], BF16, name="p_sb")
            nc.sync.dma_start(out=p_sb[:], in_=p_my.rearrange("o (j r) -> r (o j)", r=128))
            ps2 = mw.tile([128, 8], BF16, name="ps2")
            nc.scalar.activation(ps2[:], p_sb[:], AF.Silu)

            w1m_sb = mw.tile([128, 8, 512], BF16, name="w1m_sb")
            nc.sync.dma_start(out=w1m_sb[:], in_=mod_w1.rearrange("(k p) m -> p k m", p=128))
            b1m_sb = mw.tile([128, 4], F32, name="b1m_sb")
            nc.sync.dma_start(out=b1m_sb[:], in_=mod_b1)
            h_ps = psm.tile([128, 4], F32, name="h_ps")
            for mt in range(4):
                for kt in range(8):
                    nc.tensor.matmul(
                        h_ps[:, mt : mt + 1],
                        w1m_sb[:, kt, mt * 128 : (mt + 1) * 128],
                        ps2[:, kt : kt + 1],
                        start=(kt == 0), stop=(kt == 7),
                    )
            h_l2 = mw.tile([128, 4], BF16, name="h_l2")
            for mt in range(4):
                nc.scalar.activation(h_l2[:, mt : mt + 1], h_ps[:, mt : mt + 1],
                                     AF.Silu, bias=b1m_sb[:, mt : mt + 1])

            w2m_sb = mw.tile([128, 4, 4 * D], BF16, name="w2m_sb")
            nc.sync.dma_start(out=w2m_sb[:], in_=mod_w2m.rearrange("(k p) m -> p k m", p=128))
            b2m_sb = mw.tile([128, 24], F32, name="b2m_sb")
            nc.sync.dma_start(out=b2m_sb[:], in_=mod_b2m)
            mod_ps = psm.tile([128, 24], F32, name="mod_ps")
            for mt in range(24):
                for kt in range(4):
                    nc.tensor.matmul(
                        mod_ps[:, mt : mt + 1],
                        w2m_sb[:, kt, mt * 128 : (mt + 1) * 128],
                        h_l2[:, kt : kt + 1],
                        start=(kt == 0), stop=(kt == 3),
                    )
            nc.vector.tensor_add(mod_l2[:], mod_ps[:], b2m_sb[:])

            w2g_sb = mw.tile([128, 4, 2 * D], BF16, name="w2g_sb")
            nc.sync.dma_start(out=w2g_sb[:], in_=mod_w2g.rearrange("(k p) m -> p k m", p=128))
            b2g_sb = mw.tile([1, 2 * D], F32, name="b2g_sb")
            nc.sync.dma_start(out=b2g_sb[:], in_=mod_b2g)
            gates = mw.tile([1, 2 * D], F32, name="gates")
            for nt in range(3):
                g_ps = psg.tile([1, 512], F32, name="g_ps", tag="g_ps")
                for kt in range(4):
                    nc.tensor.matmul(
                        g_ps[:], h_l2[:, kt : kt + 1],
                        w2g_sb[:, kt, nt * 512 : (nt + 1) * 512],
                        start=(kt == 0), stop=(kt == 3),
                    )
                nc.vector.tensor_tensor(gates[:, nt * 512 : (nt + 1) * 512], g_ps[:],
                                        b2g_sb[:, nt * 512 : (nt + 1) * 512], op=ALU.add)
            nc.gpsimd.partition_broadcast(g_my_bc[:], gates[:, 0:D])
            nc.gpsimd.partition_broadcast(m3g_bc[:], gates[:, D : 2 * D])

            obg_sb = mw.tile([1, D], F32, name="obg_sb")
            nc.sync.dma_start(out=obg_sb[:], in_=ob_g)
            nc.gpsimd.partition_broadcast(ob_bc[:], obg_sb[:])
            b2r_sb = mw.tile([1, D], F32, name="b2r_sb")
            nc.sync.dma_start(out=b2r_sb[:], in_=b2r)
            nc.gpsimd.partition_broadcast(b2_bc[:], b2r_sb[:])

            n1my_sb = mw.tile([128, KT], F32, name="n1my_sb")
            n2my_sb = mw.tile([128, KT], F32, name="n2my_sb")
            nc.sync.dma_start(out=n1my_sb[:], in_=norm1_my)
            nc.sync.dma_start(out=n2my_sb[:], in_=norm2_my)
            tmp6 = mw.tile([128, KT], F32, name="tmp6")
            nc.vector.tensor_scalar_add(tmp6[:], mod_l2[:, 0:6], 1.0)
            nc.vector.tensor_mul(w1p[:], n1my_sb[:], tmp6[:])
            tmp6c = mw.tile([128, KT], F32, name="tmp6c")
            nc.vector.tensor_scalar_add(tmp6c[:], mod_l2[:, 12:18], 1.0)
            nc.vector.tensor_mul(w3p[:], n2my_sb[:], tmp6c[:])
            nc.sync.dma_start(out=s2_sb[:], in_=qk_s2)
            nc.sync.dma_start(out=bq_sb[:], in_=bq)
            nc.sync.dma_start(out=bk_sb[:], in_=bk)

        # ---------------- big persistent activations ----------------
        x1n = pp.tile([128, 4, D], F32, name="x1n")
        with tc.tile_pool(name="poolA", bufs=1) as pa:
            NR = 3 if DYN else 4
            Q_sb = pa.tile([128, H, CH], BF16, name="Q_sb")
            K_own = pa.tile([128, H, CH], BF16, name="K_own")
            V_own = pa.tile([128, 4, VW], BF16, name="V_own")
            pv_own = pa.tile([97, H, CH], F32, name="pv_own")
            K_all = pa.tile([128, NR, H, CH], BF16, name="K_all")
            V_all = pa.tile([128, NR, 4, VW], BF16, name="V_all")

            # ---------------- phase 1: xm + Q/K/V + rope + qk-norm fold ----------------
            with (
                nc.named_scope("proj"),
                tc.tile_pool(name="wp", bufs=1) as wp,
                tc.tile_pool(name="ph1", bufs=2) as ph1,
                tc.tile_pool(name="psP", bufs=2, space="PSUM") as psP,
                tc.tile_pool(name="psV", bufs=2, space="PSUM") as psV,
                tc.tile_pool(name="psT", bufs=2, space="PSUM") as psT,
            ):
                # rms stats (x stays raw; normalized copies per token-tile)
                ssq4 = ph1.tile([128, 4], F32, name="ssq4", tag="ssq4")
                for tt in range(4):
                    sq = ph1.tile([128, D], F32, name="sq", tag="sq")
                    nc.scalar.activation(sq[:], x_l1[:, tt, :], AF.Square,
                                         accum_out=ssq4[:, tt : tt + 1])
                rstd4 = ph1.tile([128, 4], F32, name="rstd4", tag="rstd4")
                nc.scalar.activation(rstd4[:], ssq4[:], AF.Abs_reciprocal_sqrt,
                                     scale=1.0 / D, bias=eps_sb[:])

                xm_bf = wp.tile([128, KT, CH], BF16, name="xm_bf")
                for tt in range(4):
                    xs = ph1.tile([128, D], F32, name="xs", tag="xs")
                    nc.vector.tensor_scalar_mul(xs[:], x_l1[:, tt, :],
                                                rstd4[:, tt : tt + 1])
                    for ft in range(KT):
                        tp = psT.tile([128, 128], F32, name="tp", tag="tp")
                        nc.tensor.transpose(tp[:], xs[:, ft * 128 : (ft + 1) * 128], ident[:])
                        nc.vector.tensor_scalar(
                            xm_bf[:, ft, tt * 128 : (tt + 1) * 128], tp[:],
                            w1p[:, ft : ft + 1], mod_l2[:, 6 + ft : 6 + ft + 1],
                            op0=ALU.mult, op1=ALU.add,
                        )

                # K projection first (gates the collective)
                wk_sb = wp.tile([128, KT, KW], BF16, name="wk_sb", tag="wqk")
                nc.sync.dma_start(out=wk_sb[:], in_=wk.rearrange("(k p) m -> p k m", p=128))
                for h in range(H):
                    kp = psP.tile([128, CH], F32, name="kp", tag="qp")
                    for kt in range(KT):
                        nc.tensor.matmul(
                            kp[:], wk_sb[:, kt, h * PH : (h + 1) * PH],
                            xm_bf[:, kt, :], start=(kt == 0), stop=(kt == KT - 1),
                        )
                    nc.vector.tensor_scalar_add(K_own[:, h, :], kp[:],
                                                bk_sb[:, h : h + 1])

                # rk = rsqrt(dh * |k|^2) folded into K columns pre-rope
                # (norm and column scaling are rope-invariant).
                for h in range(H):
                    ksq = ph1.tile([128, CH], BF16, name="ksq", tag="ksq")
                    nc.vector.tensor_mul(ksq[:], K_own[:, h, :], K_own[:, h, :])
                    rk_ps = psP.tile([128, CH], F32, name="rk_ps", tag="qp")
                    nc.tensor.matmul(rk_ps[0:1, :], ones_bf[:], ksq[:], start=True, stop=True)
                    rk_row = ph1.tile([1, CH], BF16, name="rk_row", tag="rk_row")
                    nc.scalar.activation(rk_row[:], rk_ps[0:1, :], AF.Abs_reciprocal_sqrt,
                                         scale=float(DH), bias=eps_sb[0:1, :])
                    rk_bc = ph1.tile([128, CH], BF16, name="rk_bc", tag="rk_bc")
                    nc.gpsimd.partition_broadcast(rk_bc[:], rk_row[:])
                    nc.vector.tensor_tensor(K_own[:, h, :], K_own[:, h, :],
                                            rk_bc[:], op=ALU.mult)

                _rope_inplace(nc, K_own[:], cos_sb[:], sin_sb[:], ph1, H)
                nc.sync.dma_start(out=k_stage, in_=K_own[:])

                # V projection
                wv_sb = wp.tile([128, KT, VW], BF16, name="wv_sb", tag="wv")
                nc.sync.dma_start(out=wv_sb[:], in_=wv.rearrange("(k p) m -> p k m", p=128))
                bv_bc = wp.tile([128, VW], F32, name="bv_bc")
                bv_row = wp.tile([1, VW], F32, name="bv_row")
                nc.sync.dma_start(out=bv_row[:], in_=bv)
                nc.gpsimd.partition_broadcast(bv_bc[:], bv_row[:])
                for tt in range(4):
                    vp1 = psV.tile([128, 512], F32, name="vp1", tag="vp1")
                    vp2 = psV.tile([128, VW - 512], F32, name="vp2", tag="vp2")
                    for kt in range(KT):
                        nc.tensor.matmul(
                            vp1[:], xm_bf[:, kt, tt * 128 : (tt + 1) * 128],
                            wv_sb[:, kt, 0:512], start=(kt == 0), stop=(kt == KT - 1),
                        )
                    for kt in range(KT):
                        nc.tensor.matmul(
                            vp2[:], xm_bf[:, kt, tt * 128 : (tt + 1) * 128],
                            wv_sb[:, kt, 512:VW], start=(kt == 0), stop=(kt == KT - 1),
                        )
                    nc.vector.tensor_tensor(V_own[:, tt, 0:512], vp1[:],
                                            bv_bc[:, 0:512], op=ALU.add)
                    nc.vector.tensor_tensor(V_own[:, tt, 512:VW], vp2[:],
                                            bv_bc[:, 512:VW], op=ALU.add)
                nc.sync.dma_start(out=v_stage, in_=V_own[:])

                # collectives (gpsimd queue; overlap Q-side work below)
                nc.gpsimd.collective_compute(
                    "AllGather", mybir.AluOpType.bypass,
                    replica_groups=RG, ins=[v_stage], outs=[v_gath],
                )
                nc.gpsimd.collective_compute(
                    "AllGather", mybir.AluOpType.bypass,
                    replica_groups=RG, ins=[k_stage], outs=[k_gath],
                )

                # Q projection + rope + s^2/|q| fold
                wq_sb = wp.tile([128, KT, KW], BF16, name="wq_sb", tag="wqk2")
                nc.sync.dma_start(out=wq_sb[:], in_=wq.rearrange("(k p) m -> p k m", p=128))
                for h in range(H):
                    qp = psP.tile([128, CH], F32, name="qp", tag="qp")
                    for kt in range(KT):
                        nc.tensor.matmul(
                            qp[:], wq_sb[:, kt, h * PH : (h + 1) * PH],
                            xm_bf[:, kt, :], start=(kt == 0), stop=(kt == KT - 1),
                        )
                    nc.vector.tensor_scalar_add(Q_sb[:, h, :], qp[:],
                                                bq_sb[:, h : h + 1])
                _rope_inplace(nc, Q_sb[:], cos_sb[:], sin_sb[:], ph1, H)
                for h in range(H):
                    qsq = ph1.tile([128, CH], BF16, name="qsq", tag="ksq")
                    nc.vector.tensor_mul(qsq[:], Q_sb[:, h, :], Q_sb[:, h, :])
                    rq_ps = psP.tile([128, CH], F32, name="rq_ps", tag="qp")
                    nc.tensor.matmul(rq_ps[0:1, :], ones_bf[:], qsq[:], start=True, stop=True)
                    rq_row = ph1.tile([1, CH], BF16, name="rq_row", tag="rk_row")
                    nc.scalar.activation(rq_row[:], rq_ps[0:1, :], AF.Abs_reciprocal_sqrt,
                                         bias=eps_sb[0:1, :])
                    rq_bc = ph1.tile([128, CH], BF16, name="rq_bc", tag=f"rk_bc{h % 2}")
                    nc.gpsimd.partition_broadcast(rq_bc[:], rq_row[:])
                    nc.vector.scalar_tensor_tensor(
                        Q_sb[:, h, :], Q_sb[:, h, :], s2_sb[:], rq_bc[:],
                        op0=ALU.mult, op1=ALU.mult,
                    )

            # load-back of the gathered K/V: only the 3 remote slots (row
            # offsets from the per-core slots input) when DYN, else all 4.
            if DYN:
                row_regs = []
                for m in range(NR):
                    reg = nc.sync.alloc_register(f"slotrow{m}")
                    nc.sync.reg_load(reg, slot_sb[0:1, m + 1 : m + 2])
                    row_regs.append(nc.snap(reg, min_val=0, max_val=3 * 128))
                rows = [bass.ds(row_regs[m], 128) for m in range(NR)]
            else:
                rows = [slice(g * 128, (g + 1) * 128) for g in range(4)]
            for m in range(NR):
                nc.sync.dma_start(
                    out=V_all[:, m], in_=v_gath[rows[m], :]
                    .rearrange("p (t c) -> p t c", t=4))
            for m in range(NR):
                nc.sync.dma_start(
                    out=K_all[:, m], in_=k_gath[rows[m], :]
                    .rearrange("p (h t) -> p h t", h=H))

            # ---------------- phase 2: attention ----------------
            with (
                tc.tile_pool(name="ph2", bufs=2) as ph2,
                tc.tile_pool(name="ph2s", bufs=1) as ph2s,
                tc.tile_pool(name="ptp", bufs=3) as ptp,
                tc.tile_pool(name="ph3w", bufs=1) as ph3w,
            ):
              with (
                nc.named_scope("attn"),
                tc.tile_pool(name="psS", bufs=3, space="PSUM") as psS,
                tc.tile_pool(name="psPV", bufs=2, space="PSUM") as psPV,
              ):
                # own-chunk scores/exp/PV from local tiles (overlaps collectives)
                if DYN:
                    for h in range(H):
                        pvo = psPV.tile([128, CH], F32, name="pvo", tag="pv")
                        for half in range(2):
                            sps = psS.tile([128, 2, CH], F32, name="sps", tag="sps")
                            for j in range(2):
                                t2 = half * 2 + j
                                nc.tensor.matmul(
                                    sps[:, j, :],
                                    K_own[:, h, t2 * 128 : (t2 + 1) * 128],
                                    Q_sb[:, h, :], start=True, stop=True,
                                )
                            pt = ptp.tile([128, 2, CH], BF16, name="pt", tag="pt")
                            nc.scalar.activation(pt[:], sps[:], AF.Exp)
                            for j in range(2):
                                t2 = half * 2 + j
                                nc.tensor.matmul(
                                    pvo[0:97, :],
                                    V_own[:, t2, h * 97 : (h + 1) * 97],
                                    pt[:, j, :],
                                    start=(half == 0 and j == 0),
                                    stop=(half == 1 and j == 1),
                                )
                        nc.vector.tensor_copy(pv_own[:, h, :], pvo[0:97, :])

                wo_sb = ph3w.tile([96, H, D], BF16, name="wo_sb")
                nc.sync.dma_start(out=wo_sb[:], in_=wo.rearrange("p (h m) -> p h m", h=H))

                attnn = ph2s.tile([96, H, CH], BF16, name="attnn")
                for h in range(H):
                    pv = psPV.tile([128, CH], F32, name="pv", tag="pv")
                    for mi in range(NR):
                        for half in range(2):
                            sps = psS.tile([128, 2, CH], F32, name="sps", tag="sps")
                            for j in range(2):
                                t2 = half * 2 + j
                                nc.tensor.matmul(
                                    sps[:, j, :],
                                    K_all[:, mi, h, t2 * 128 : (t2 + 1) * 128],
                                    Q_sb[:, h, :], start=True, stop=True,
                                )
                            pt = ptp.tile([128, 2, CH], BF16, name="pt", tag="pt")
                            nc.scalar.activation(pt[:], sps[:], AF.Exp)
                            for j in range(2):
                                t2 = half * 2 + j
                                last = (mi == NR - 1) and (half == 1) and (j == 1)
                                nc.tensor.matmul(
                                    pv[0:97, :],
                                    V_all[:, mi, t2, h * 97 : (h + 1) * 97],
                                    pt[:, j, :],
                                    start=mi == 0 and half == 0 and j == 0,
                                    stop=last,
                                )
                    tot = ph2.tile([97, CH], F32, name="tot", tag="tot")
                    if DYN:
                        nc.vector.tensor_tensor(tot[:], pv[0:97, :], pv_own[:, h, :],
                                                op=ALU.add)
                    else:
                        nc.vector.tensor_copy(tot[:], pv[0:97, :])
                    rs2 = ph2.tile([1, CH], F32, name="rs2", tag="rs2")
                    nc.scalar.activation(rs2[:], tot[96:97, :], AF.Square)
                    rs = ph2.tile([1, CH], F32, name="rs", tag="rs")
                    nc.scalar.activation(rs[:], rs2[:], AF.Abs_reciprocal_sqrt)
                    rs_bc = ph2.tile([96, CH], F32, name="rs_bc", tag="rs_bc")
                    nc.gpsimd.partition_broadcast(rs_bc[:], rs[:], channels=96)
                    nc.vector.tensor_tensor(attnn[:, h, :], tot[0:96, :], rs_bc[:], op=ALU.mult)

              # ---------------- phase 3: out-proj + residual ----------------
              with (
                    nc.named_scope("oproj"),
                    tc.tile_pool(name="psO", bufs=2, space="PSUM") as psO,
              ):
                    for qt in range(4):
                        op1 = psO.tile([128, 512], F32, name="op1", tag="op1")
                        op2 = psO.tile([128, D - 512], F32, name="op2", tag="op2")
                        for h in range(H):
                            nc.tensor.matmul(
                                op1[:], attnn[:, h, qt * 128 : (qt + 1) * 128],
                                wo_sb[:, h, 0:512], start=(h == 0), stop=(h == H - 1),
                            )
                        for h in range(H):
                            nc.tensor.matmul(
                                op2[:], attnn[:, h, qt * 128 : (qt + 1) * 128],
                                wo_sb[:, h, 512:D], start=(h == 0), stop=(h == H - 1),
                            )
                        t1 = ph2.tile([128, D], F32, name="t1", tag="t1")
                        nc.vector.tensor_tensor(t1[:, 0:512], op1[:], ob_bc[:, 0:512], op=ALU.add)
                        nc.vector.tensor_tensor(t1[:, 512:D], op2[:], ob_bc[:, 512:D], op=ALU.add)
                        nc.vector.tensor_mul(t1[:], t1[:], g_my_bc[:])
                        nc.vector.tensor_tensor(x1n[:, qt, :], t1[:], x_l1[:, qt, :], op=ALU.add)

        # ---------------- phase 4: norm2 + MLP + final ----------------
        with (
            nc.named_scope("mlp"),
            tc.tile_pool(name="ph4", bufs=2) as ph4,
            tc.tile_pool(name="mlpw", bufs=1) as mlpw,
            tc.tile_pool(name="psM", bufs=2, space="PSUM") as psM,
            tc.tile_pool(name="psM2", bufs=2, space="PSUM") as psM2,
            tc.tile_pool(name="psT2", bufs=2, space="PSUM") as psT2,
        ):
            w1_sb = mlpw.tile([128, KT, MLPD], BF16, name="w1_sb")
            nc.gpsimd.dma_start(out=w1_sb[:], in_=w1.rearrange("(k p) m -> p k m", p=128))
            w2_sb = mlpw.tile([128, MT2, D], BF16, name="w2_sb")
            nc.gpsimd.dma_start(out=w2_sb[:], in_=w2.rearrange("(k p) m -> p k m", p=128))
            b1_sb = mlpw.tile([128, MT2], F32, name="b1_sb")
            nc.sync.dma_start(out=b1_sb[:], in_=b1c)

            xn_l2 = mlpw.tile([128, KT, CH], BF16, name="xn_l2")
            ssq4m = ph4.tile([128, 4], F32, name="ssq4m", tag="ssq4m")
            for tt in range(4):
                sq = ph4.tile([128, D], F32, name="sq4", tag="sq4")
                nc.scalar.activation(sq[:], x1n[:, tt, :], AF.Square,
                                     accum_out=ssq4m[:, tt : tt + 1])
            rstd4m = ph4.tile([128, 4], F32, name="rstd4m", tag="rstd4m")
            nc.scalar.activation(rstd4m[:], ssq4m[:], AF.Abs_reciprocal_sqrt,
                                 scale=1.0 / D, bias=eps_sb[:])
            for tt in range(4):
                xs = ph4.tile([128, D], F32, name="xs4", tag="xs4")
                nc.vector.tensor_scalar_mul(xs[:], x1n[:, tt, :], rstd4m[:, tt : tt + 1])
                for ft in range(KT):
                    tp = psT2.tile([128, 128], F32, name="tp2", tag="tp2")
                    nc.tensor.transpose(tp[:], xs[:, ft * 128 : (ft + 1) * 128], ident[:])
                    nc.vector.tensor_scalar(
                        xn_l2[:, ft, tt * 128 : (tt + 1) * 128], tp[:],
                        w3p[:, ft : ft + 1], mod_l2[:, 18 + ft : 18 + ft + 1],
                        op0=ALU.mult, op1=ALU.add,
                    )

            h_bf = mlpw.tile([128, MT2, CH], BF16, name="h_bf")
            for mt in range(MT2):
                fp = psM.tile([128, CH], F32, name="fp", tag="fp")
                for kt in range(KT):
                    nc.tensor.matmul(
                        fp[:], w1_sb[:, kt, mt * 128 : (mt + 1) * 128],
                        xn_l2[:, kt, :], start=(kt == 0), stop=(kt == KT - 1),
                    )
                nc.scalar.activation(h_bf[:, mt, :], fp[:], AF.Gelu,
                                     bias=b1_sb[:, mt : mt + 1])

            out_f = mlpw.tile([128, 4, D], F32, name="out_f")
            for qt in range(4):
                f1 = psM2.tile([128, 512], F32, name="f1", tag="f1")
                f2 = psM2.tile([128, D - 512], F32, name="f2", tag="f2")
                for mt in range(MT2):
                    nc.tensor.matmul(
                        f1[:], h_bf[:, mt, qt * 128 : (qt + 1) * 128],
                        w2_sb[:, mt, 0:512], start=(mt == 0), stop=(mt == MT2 - 1),
                    )
                for mt in range(MT2):
                    nc.tensor.matmul(
                        f2[:], h_bf[:, mt, qt * 128 : (qt + 1) * 128],
                        w2_sb[:, mt, 512:D], start=(mt == 0), stop=(mt == MT2 - 1),
                    )
                t2 = ph4.tile([128, D], F32, name="t2", tag="t2")
                nc.vector.tensor_tensor(t2[:, 0:512], f1[:], b2_bc[:, 0:512], op=ALU.add)
                nc.vector.tensor_tensor(t2[:, 512:D], f2[:], b2_bc[:, 512:D], op=ALU.add)
                nc.vector.tensor_mul(t2[:], t2[:], m3g_bc[:])
                nc.vector.tensor_tensor(out_f[:, qt, :], t2[:], x1n[:, qt, :], op=ALU.add)
            nc.sync.dma_start(out=my_out.rearrange("(t p) c -> p t c", p=128), in_=out_f[:])

        persist_cm.__exit__(None, None, None)

    nc.compile()
    _CACHED["nc"] = nc
    return nc


def _pad_head_cols(w_h, b_h):
    wp = np.zeros((D, PH), np.float32)
    bp = np.zeros((PH,), np.float32)
    wp[:, _ROWS_LO] = w_h[:, 0:48]
    wp[:, _ROWS_HI] = w_h[:, 48:96]
    bp[_ROWS_LO] = b_h[0:48]
    bp[_ROWS_HI] = b_h[48:96]
    return wp, bp


def _prep_core_inputs(c, inp):
    b, r = c // 4, c % 4
    s, sub = r // 2, r % 2

    x1 = np.asarray(inp["x_stream1"], np.float32)
    x2 = np.asarray(inp["x_stream2"], np.float32)
    my = [x1, x2][s][b]
    x_own = np.ascontiguousarray(my[sub * CH : (sub + 1) * CH])

    pos = np.arange(r * CH, (r + 1) * CH)
    inv = (1.0 / (10000.0 ** (np.arange(0, DH, 2, dtype=np.float32) / DH)))
    inv = inv.astype(_BF16).astype(np.float32)
    freqs = pos[:, None].astype(np.float32) * inv[None, :]
    emb = np.concatenate([freqs, freqs], axis=-1)
    cos_d, sin_d = np.cos(emb), np.sin(emb)
    cos_p = np.zeros((128, CH), np.float32)
    sin_p = np.zeros((128, CH), np.float32)
    cos_p[_ROWS_LO] = cos_d[:, 0:48].T
    cos_p[_ROWS_HI] = cos_d[:, 48:96].T
    sin_p[_ROWS_LO] = sin_d[:, 48:96].T
    sin_p[_ROWS_HI] = sin_d[:, 0:48].T

    qkv_w = [np.asarray(inp["qkv_w"], np.float32), np.asarray(inp["qkv2_w"], np.float32)]
    qkv_b = [np.asarray(inp["qkv_b"], np.float32), np.asarray(inp["qkv2_b"], np.float32)]

    def padded(part):
        wfull = qkv_w[s][:, part * D : (part + 1) * D]
        bfull = qkv_b[s][part * D : (part + 1) * D]
        wp = np.zeros((D, H * PH), np.float32)
        bp = np.zeros((128, H), np.float32)
        for h in range(H):
            whp, bhp = _pad_head_cols(wfull[:, h * DH : (h + 1) * DH],
                                      bfull[h * DH : (h + 1) * DH])
            wp[:, h * PH : (h + 1) * PH] = whp
            bp[:, h] = bhp
        return wp.astype(_BF16), bp

    wq_p, bq_p = padded(0)
    wk_p, bk_p = padded(1)

    wfull = qkv_w[s][:, 2 * D : 3 * D]
    bfull = qkv_b[s][2 * D : 3 * D]
    wv_a = np.zeros((D, VW), np.float32)
    bv_a = np.zeros((1, VW), np.float32)
    for h in range(H):
        wv_a[:, h * 97 : h * 97 + 96] = wfull[:, h * DH : (h + 1) * DH]
        bv_a[0, h * 97 : h * 97 + 96] = bfull[h * DH : (h + 1) * DH]
        bv_a[0, h * 97 + 96] = 1.0

    qs = np.asarray(inp["qk_scale"], np.float32)
    s2 = np.zeros((128, 1), np.float32)
    s2[_ROWS_LO, 0] = qs[0:48] ** 2
    s2[_ROWS_HI, 0] = qs[48:96] ** 2

    def l2cols(v):
        return np.ascontiguousarray(np.asarray(v, np.float32).reshape(KT, 128).T)

    ms_my, mh_my, g_my = (0, 1, 2) if s == 0 else (3, 4, 5)
    m3s, m3h, m3g = (6, 7, 8) if s == 0 else (9, 10, 11)

    w2f = np.asarray(inp["mod_w2"], np.float32)
    b2f = np.asarray(inp["mod_b2"], np.float32)
    cw = lambda i: w2f[:, i * D : (i + 1) * D]
    cb = lambda i: b2f[i * D : (i + 1) * D]
    main_idx = [ms_my, mh_my, m3s, m3h]
    mod_w2m = np.concatenate([cw(i) for i in main_idx], axis=1).astype(_BF16)
    mod_b2m = np.ascontiguousarray(np.concatenate([l2cols(cb(i)) for i in main_idx], axis=1))
    mod_w2g = np.concatenate([cw(g_my), cw(m3g)], axis=1).astype(_BF16)
    mod_b2g = np.ascontiguousarray(np.concatenate([cb(g_my), cb(m3g)])[None, :])

    wo_f = np.asarray(inp["out_w"], np.float32)
    wo_dev = np.ascontiguousarray(wo_f.reshape(H, DH, D).transpose(1, 0, 2).reshape(DH, H * D))

    norm1 = [np.asarray(inp["norm11_w"], np.float32), np.asarray(inp["norm12_w"], np.float32)]
    norm2 = [np.asarray(inp["norm21_w"], np.float32), np.asarray(inp["norm22_w"], np.float32)]
    mlw = [
        (inp["mlp1_w1"], inp["mlp1_b1"], inp["mlp1_w2"], inp["mlp1_b2"]),
        (inp["mlp2_w1"], inp["mlp2_b1"], inp["mlp2_w2"], inp["mlp2_b2"]),
    ]
    w1f, b1f, w2mf, b2mf = [np.asarray(a, np.float32) for a in mlw[s]]

    slots_v = np.array(
        [[r * 128, ((r + 1) % 4) * 128, ((r + 2) % 4) * 128, ((r + 3) % 4) * 128]],
        np.int32)

    return {
        "x_own": x_own,
        "p_my": np.asarray(inp["p_emb"], np.float32)[b].astype(_BF16),
        "mod_w1": np.asarray(inp["mod_w1"], np.float32).astype(_BF16),
        "mod_b1": np.ascontiguousarray(np.asarray(inp["mod_b1"], np.float32).reshape(4, 128).T),
        "mod_w2m": mod_w2m,
        "mod_b2m": mod_b2m,
        "mod_w2g": mod_w2g,
        "mod_b2g": mod_b2g,
        "norm1_my": l2cols(norm1[s]),
        "norm2_my": l2cols(norm2[s]),
        "wq": wq_p, "bq": bq_p,
        "wk": wk_p, "bk": bk_p,
        "wv": wv_a.astype(_BF16), "bv": bv_a,
        "cos_t": cos_p.astype(_BF16), "sin_t": sin_p.astype(_BF16), "qk_s2": s2,
        "wo": wo_dev.astype(_BF16),
        "ob_g": np.ascontiguousarray(np.asarray(inp["out_b"], np.float32)[None, :]),
        "w1": w1f.astype(_BF16),
        "b1c": np.ascontiguousarray(b1f.reshape(MT2, 128).T),
        "w2": w2mf.astype(_BF16),
        "b2r": np.ascontiguousarray(b2mf[None, :]),
        "slots": slots_v,
    }


def kernel(**inputs):
    nc = _build()
    in_maps = [_prep_core_inputs(c, inputs) for c in range(NC)]
    res = run_bass_kernel_spmd(nc, in_maps, core_ids=list(range(NC)), trace=False)
    out1 = np.zeros((B, T, D), np.float32)
    out2 = np.zeros((B, T, D), np.float32)
    for c in range(NC):
        b, r = c // 4, c % 4
        dst = out1 if r < 2 else out2
        sub = r % 2
        dst[b, sub * CH : (sub + 1) * CH] = res.results[c]["my_out"]
    return out1, out2
